# revision 39
# baseline (speedup 1.0000x reference)
"""Trainium2 Bass kernel for nn_Explainer (gnn_message_passing).

Math (reference):
  f12[i*n+j] = concat(embed[i], embed[j]);  h = relu(f12 @ W1 + b1)
  log_alpha = h @ W2 + b2
  gate = sigmoid((log(u) - log(1-u) + log_alpha) / beta)
  sym = (gate + gate.T)/2 ; masked = adj * sym
  hg = relu((masked @ x) @ Wg1); pooled = hg.mean(0); softmax(pooled @ Wg2)

Key decomposition: f12 @ W1 + b1 = A[i] + B[j] with
  A = embed @ W1[:64] + b1   (per-row), B = embed @ W1[64:]
so log_alpha[i,j] = W2 . relu(A[i] + B[j]) + b2 -- no [N^2,128] matmul needed.

Sharding: row-blocks of the i dimension across 8 cores. The pre-sigmoid
log-odds (pre = nl + la) are exchanged with two AllToAlls in bf16 (declared
f32 via bitcast: the CC rate is per element and bypass A2A is a byte move);
both sides apply the sigmoid locally. The masked GNN contraction is split
  masked @ x = (adj*gateT_own) @ x + (adj*gate_recv) @ x
so the own-gate term runs while the AllToAll is in flight, and the received
term consumes the A2A output in its NATURAL packed layout (one fat
128x1KB DMA, no 512-descriptor scatter): host-side x/adj replicas in the
same packed layout turn the layout mismatch into 4 extra matmuls per half.
Per-core class logits are combined with a tiny AllToAll + ones-matmul and
the softmax is replicated.

The CC engine has a ~21+(24..52)us init barrier (varies run to run, likely
neighbor congestion); the first collective's data starts at
max(barrier_end, trigger) + ~11us and later ones chain at ~1.8us gaps. The
design keeps everything before the barrier end off the critical path and
minimizes the post-barrier serial chain.
"""
import numpy as np
import ml_dtypes

import concourse.bass as bass
import concourse.bacc as bacc
import concourse.tile as tile
from concourse import mybir
from concourse.bass_utils import run_bass_kernel_spmd

N = 1024
NC = 8
R = N // NC          # 128 rows per core
D = 64               # embed dim
H = 64               # hidden
F = 128              # x features
C = 8                # classes
NPAIR = R // 2       # 64 i-pairs per core
GRP = 16             # pairs per PE column-group / psum row-group
NCHUNK = NPAIR // GRP  # 4 row-chunks of 32 pre rows

F32 = mybir.dt.float32
BF16 = mybir.dt.bfloat16
MM_DT = BF16
DEBUG_OUTPUTS = False


def _mask_w2_np():
    """[128, NPAIR, 32] mask: 1.0 where the block-diag W2 stack has W2 values.

    Pair t -> psum row-group g=t//16 (tile_position=(0,32g)), slot s=t%16.
    lhsT_t = W2S[:, t, :]: col 2s rows 0:64 = W2, col 2s+1 rows 64:128 = W2.
    psum out row for pair t = 32g + 2s (+1) = 2t (+1) = local i'.
    """
    cols = 32
    m = np.zeros((128, NPAIR, cols), np.float32)
    for t in range(NPAIR):
        s = t % GRP
        m[0:64, t, 2 * s] = 1.0
        m[64:128, t, 2 * s + 1] = 1.0
    return m


def _fat_jmap(h):
    """Packed-layout j index for A2A half h: natFat[p, k] holds shard row
    4p+k of a2a_out[h], i.e. sender r=p//16's pre row m=4*(p%16)+k, which
    is global row j = 128r + 64h + m."""
    p = np.arange(128)[:, None]
    k = np.arange(4)[None, :]
    return 128 * (p // 16) + 64 * h + 4 * (p % 16) + k  # [128, 4]


def build():
    nc = bacc.Bacc("TRN2", target_bir_lowering=False, debug=False, num_devices=NC)

    # ---- kernel I/O ----
    embT_in = nc.dram_tensor("embT_in", [D, N], F32, kind="ExternalInput")
    embTs_in = nc.dram_tensor("embTs_in", [D, R], F32, kind="ExternalInput")
    xbf_in = nc.dram_tensor("xbf_in", [128, NC, F], BF16, kind="ExternalInput")
    adjT_in = nc.dram_tensor("adjT_in", [128, NC, 128], BF16,
                             kind="ExternalInput")
    xfat_in = nc.dram_tensor("xfat_in", [2, 128, 4, F], BF16,
                             kind="ExternalInput")
    afat_in = nc.dram_tensor("afat_in", [2, 128, 4, 128], BF16,
                             kind="ExternalInput")
    noise_slab = nc.dram_tensor("noise_slab", [R, N], F32, kind="ExternalInput")
    tmp_in = nc.dram_tensor("tmp_in", [1, 1], F32, kind="ExternalInput")
    w1_in = nc.dram_tensor("w1_in", [2 * D, H], F32, kind="ExternalInput")
    b1_in = nc.dram_tensor("b1_in", [1, H], F32, kind="ExternalInput")
    w2_in = nc.dram_tensor("w2_in", [H, 1], F32, kind="ExternalInput")
    b2_in = nc.dram_tensor("b2_in", [1, 1], F32, kind="ExternalInput")
    wg1_in = nc.dram_tensor("wg1_in", [F, H], F32, kind="ExternalInput")
    wg2_in = nc.dram_tensor("wg2_in", [H, C], F32, kind="ExternalInput")
    out_dram = nc.dram_tensor("out", [1, C], F32, kind="ExternalOutput")
    dbg = {}
    if DEBUG_OUTPUTS:
        for nm, shp in [("d_pre", [R, N]), ("d_gate", [R, N]),
                        ("d_tT", [128, 128]), ("d_pooled", [1, H])]:
            dbg[nm] = nc.dram_tensor(nm, shp, F32, kind="ExternalOutput")

    # ---- compile-time constants ----
    identb_c = nc.inline_tensor(np.eye(128, dtype=np.float32).astype(
        mybir.dt.np(BF16)), name="identb")
    maskw2_c = nc.inline_tensor(_mask_w2_np().astype(
        np.float32).astype(mybir.dt.np(MM_DT)), name="maskw2")
    ones128_c = nc.inline_tensor(np.ones((1, 128), np.float32), name="ones128")
    ones8_c = nc.inline_tensor(np.ones((8, 1), np.float32), name="ones8")

    with tile.TileContext(nc) as tc:
        with (
            tc.tile_pool(name="const", bufs=1) as constp,
            tc.tile_pool(name="big", bufs=1) as big,
            tc.tile_pool(name="tmpp", bufs=6) as tmpp,
            tc.tile_pool(name="pla", bufs=1, space="PSUM") as pla,
            tc.tile_pool(name="ptp", bufs=2, space="PSUM") as ptp,
            tc.tile_pool(name="psm", bufs=2, space="PSUM") as psm,
            tc.tile_pool(name="dram", bufs=1, space="DRAM") as dram,
        ):
            # ============ phase 0: loads + precompute =======================
            # critical-path loads on the sync (SP) queue; its dispatch does
            # not steal DVE/ACT time. The sync queue is then reserved for
            # the per-chunk staging DMAs.
            embT = big.tile([D, N], F32)
            nc.sync.dma_start(embT[:], embT_in[:])
            w1a_sb = big.tile([D, H], F32)
            nc.sync.dma_start(w1a_sb[:], w1_in[0:D, :])
            w1b_sb = big.tile([D, H], F32)
            nc.sync.dma_start(w1b_sb[:], w1_in[D:2 * D, :])
            eTs = big.tile([D, R], F32)
            nc.sync.dma_start(eTs[:], embTs_in[:])
            b1t_sb = big.tile([H, 1], F32)
            nc.sync.dma_start(b1t_sb[:], b1_in[:].rearrange("o h -> h o"))
            w2_sb = big.tile([H, 1], F32)
            nc.sync.dma_start(w2_sb[:], w2_in[:])
            noise_sb = big.tile([R, N], F32)
            nc.sync.dma_start(noise_sb[:], noise_slab[:])
            maskw2 = constp.tile([128, NPAIR, 32], MM_DT)
            nc.sync.dma_start(maskw2[:, 0:GRP, :], maskw2_c[:, 0:GRP, :])
            nc.sync.dma_start(maskw2[:, GRP:NPAIR, :],
                              maskw2_c[:, GRP:NPAIR, :])

            # GNN operands on the gpsimd queue (pre-barrier time is free;
            # the skinny-descriptor std-layout loads grind harmlessly here)
            identb = constp.tile([128, 128], BF16)
            nc.gpsimd.dma_start(identb[:], identb_c[:])
            ones128 = constp.tile([1, 128], F32)
            nc.gpsimd.dma_start(ones128[:], ones128_c[:])
            ones8 = constp.tile([8, 1], F32)
            nc.gpsimd.dma_start(ones8[:], ones8_c[:])
            b2_sb = big.tile([1, 1], F32)
            nc.gpsimd.dma_start(b2_sb[:], b2_in[:])
            tmp_sb = big.tile([1, 1], F32)
            nc.gpsimd.dma_start(tmp_sb[:], tmp_in[:])
            wg1_sb = big.tile([F, H], F32)
            nc.gpsimd.dma_start(wg1_sb[:], wg1_in[:])
            wg2_sb = big.tile([H, C], F32)
            nc.gpsimd.dma_start(wg2_sb[:], wg2_in[:])
            x_bf = big.tile([128, NC, F], BF16)
            nc.gpsimd.dma_start(x_bf[:], xbf_in[:])
            adjT_bf = big.tile([128, NC, 128], BF16)
            nc.gpsimd.dma_start(adjT_bf[:], adjT_in[:])
            x_fat = [big.tile([128, 4, F], BF16, name=f"x_fat{h}")
                     for h in range(2)]
            a_fat = [big.tile([128, 4, 128], BF16, name=f"a_fat{h}")
                     for h in range(2)]
            for h in range(2):
                nc.gpsimd.dma_start(x_fat[h][:], xfat_in[h])
                nc.gpsimd.dma_start(a_fat[h][:], afat_in[h])

            # PE warm-up: dependency-free dummy matmuls so the HAM clock-gate
            # opens (1.2 -> 2.4 GHz) before the real matmul stream starts.
            # More dummies are interleaved into the first pairs below to keep
            # PE utilization high while the early (DVE-gated) pairs trickle,
            # so HAM does not drop back to half rate mid-phase-1.
            warm_sb = tmpp.tile([128, 512], MM_DT, tag="warm")
            nc.vector.memset(warm_sb[:], 0.0)
            ones64_8 = constp.tile([64, 8], F32)
            nc.gpsimd.memset(ones64_8[:], 1.0)
            for _ in range(8):
                warm_ps = psm.tile([1, 512], F32, tag="sm", name="warm_ps")
                nc.tensor.matmul(warm_ps[:], warm_sb[:, 0:1], warm_sb[:])

            # scaled GNN weights: 0.5 into Wg1 (symmetrize), 1/1024 into Wg2
            wg1h = big.tile([F, H], BF16)
            nc.gpsimd.tensor_scalar(out=wg1h[:], in0=wg1_sb[:],
                                    scalar1=0.5, scalar2=None,
                                    op0=mybir.AluOpType.mult)
            wg2s = big.tile([H, C], F32)
            nc.gpsimd.tensor_scalar(out=wg2s[:], in0=wg2_sb[:],
                                    scalar1=1.0 / N, scalar2=None,
                                    op0=mybir.AluOpType.mult)

            # A^T for this core's slab: [64, 128] = W1a^T @ embed_slab^T + b1
            at_ps = psm.tile([H, R], F32, tag="sm")
            nc.tensor.matmul(at_ps[:], w1a_sb[:], eTs[:])
            ats = big.tile([H, R], F32)
            nc.vector.tensor_scalar(out=ats[:], in0=at_ps[:],
                                    scalar1=b1t_sb[:], scalar2=None,
                                    op0=mybir.AluOpType.add)
            atstack = big.tile([128, NPAIR], F32)
            ats_pair = ats[:].rearrange("h (t two) -> h two t", two=2)
            nc.vector.tensor_copy(atstack[0:H, :], ats_pair[:, 0, :])
            nc.vector.tensor_copy(atstack[H:128, :], ats_pair[:, 1, :])

            # B^T (full): [64, 1024], then stacked twice -> [128, 1024] bf16
            btstack = big.tile([128, N], MM_DT)
            for jc in range(2):
                bt_ps = psm.tile([H, 512], F32, tag="sm")
                nc.tensor.matmul(bt_ps[:], w1b_sb[:],
                                 embT[:, jc * 512:(jc + 1) * 512])
                nc.vector.tensor_copy(
                    btstack[0:H, jc * 512:(jc + 1) * 512], bt_ps[:])
                nc.scalar.copy(
                    btstack[H:128, jc * 512:(jc + 1) * 512], bt_ps[:])

            # W2 stacks: maskw2 * [W2; W2] per-partition (group 0 first)
            w2col = big.tile([128, 1], F32)
            nc.vector.tensor_copy(w2col[0:H, :], w2_sb[:])
            nc.vector.tensor_copy(w2col[H:128, :], w2_sb[:])
            w2s_t = big.tile([128, NPAIR, 32], MM_DT)
            nc.vector.tensor_scalar(
                out=w2s_t[:, 0:GRP, :].rearrange("p t c -> p (t c)"),
                in0=maskw2[:, 0:GRP, :].rearrange("p t c -> p (t c)"),
                scalar1=w2col[:], scalar2=None, op0=mybir.AluOpType.mult)
            nc.vector.tensor_scalar(
                out=w2s_t[:, GRP:NPAIR, :].rearrange("p t c -> p (t c)"),
                in0=maskw2[:, GRP:NPAIR, :].rearrange("p t c -> p (t c)"),
                scalar1=w2col[:], scalar2=None, op0=mybir.AluOpType.mult)

            # gate scale/bias: sigmoid(invb * pre + invb*b2)
            invb = big.tile([1, 1], F32)
            nc.vector.reciprocal(invb[:], tmp_sb[:])
            ib2 = big.tile([1, 1], F32)
            nc.vector.tensor_tensor(ib2[:], invb[:], b2_sb[:],
                                    op=mybir.AluOpType.mult)
            invb_ps = psm.tile([128, 1], F32, tag="sm")
            nc.tensor.matmul(invb_ps[:], ones128[:], invb[:])
            invb128 = big.tile([128, 1], F32)
            nc.vector.tensor_copy(invb128[:], invb_ps[:])
            ib2_ps = psm.tile([128, 1], F32, tag="sm")
            nc.tensor.matmul(ib2_ps[:], ones128[:], ib2[:])
            ib2b = big.tile([128, 1], F32)
            nc.vector.tensor_copy(ib2b[:], ib2_ps[:])

            # noise transform on ACT: nl = ln(u) - ln(1-u)
            logu = big.tile([R, N], F32)
            nc.scalar.activation(logu[:], noise_sb[:],
                                 mybir.ActivationFunctionType.Ln)
            log1mu = big.tile([R, N], F32)
            nc.scalar.activation(log1mu[:], noise_sb[:],
                                 mybir.ActivationFunctionType.Ln,
                                 bias=1.0, scale=-1.0)
            nl = big.tile([R, N], F32)
            nc.vector.tensor_tensor(nl[:], logu[:], log1mu[:],
                                    op=mybir.AluOpType.subtract)

            # ========= phase 1: edge MLP, pre staged per psum group =========
            la_ps = [pla.tile([128, 512], F32, tag=f"la{jc}", name=f"la_ps{jc}")
                     for jc in range(2)]
            pre = big.tile([R, N], BF16)
            a2a_in = [dram.tile([N // 2, 128], BF16, name=f"a2a_in{h}")
                      for h in range(2)]
            a2a_out = [dram.tile([N // 2, 128], BF16, name=f"a2a_out{h}")
                       for h in range(2)]
            a2a_in_r = [a2a_in[h][:].rearrange("(r m) i -> m r i", r=NC)
                        for h in range(2)]
            for t in range(NPAIR):
                g, s = t // GRP, t % GRP
                tmpb = tmpp.tile([128, N], MM_DT, tag="relu")
                if t % 3 == 2:
                    nc.scalar.activation(
                        tmpb[:], btstack[:],
                        mybir.ActivationFunctionType.Relu,
                        bias=atstack[:, t:t + 1])
                else:
                    nc.vector.tensor_scalar(
                        out=tmpb[:], in0=btstack[:],
                        scalar1=atstack[:, t:t + 1], scalar2=0.0,
                        op0=mybir.AluOpType.add, op1=mybir.AluOpType.max)
                for jc in range(2):
                    nc.tensor.matmul(
                        la_ps[jc][32 * g:32 * (g + 1), :],
                        w2s_t[:, t, :],
                        tmpb[:, jc * 512:(jc + 1) * 512],
                        start=(s == 0), stop=(s == GRP - 1),
                        tile_position=(0, 32 * g))
                if t < 8:
                    warm_ps = psm.tile([1, 512], F32, tag="sm",
                                       name=f"warm_ps_{t}")
                    nc.tensor.matmul(warm_ps[:], warm_sb[:, 0:1], warm_sb[:])
                if s == GRP - 1:
                    # group g's psum rows are final: pre chunk + stage
                    lo, hi = 32 * g, 32 * (g + 1)
                    for jc in range(2):
                        nc.vector.tensor_tensor(
                            pre[lo:hi, jc * 512:(jc + 1) * 512],
                            la_ps[jc][lo:hi, :],
                            nl[lo:hi, jc * 512:(jc + 1) * 512],
                            op=mybir.AluOpType.add)
                    h = g // 2
                    nc.sync.dma_start(
                        a2a_in_r[h][lo - 64 * h:hi - 64 * h],
                        pre[lo:hi, :].rearrange("m (r i) -> m r i", r=NC))
                    if g % 2 == 1:
                        # half h fully staged: ring its AllToAll. bf16
                        # payload DECLARED f32 (bitcast): the CC rate is per
                        # element and bypass A2A is a pure byte move. Two
                        # halves beat one 256KB op (less congestion-prone)
                        # and four quarters (CC ops chain at +1.8us each).
                        nc.gpsimd.collective_compute(
                            "AllToAll", mybir.AluOpType.bypass,
                            replica_groups=[list(range(NC))],
                            ins=[a2a_in[h][:].bitcast(F32).opt()],
                            outs=[a2a_out[h][:].bitcast(F32).opt()])

            # own gate + transposes + the own-gate GNN term, all while the
            # AllToAlls are in flight
            gate = big.tile([R, N], BF16)
            nc.scalar.activation(gate[:], pre[:],
                                 mybir.ActivationFunctionType.Sigmoid,
                                 bias=ib2b[:], scale=invb128[:])
            gTc = big.tile([128, NC, 128], BF16)
            for r in range(NC):
                pt = ptp.tile([128, 128], BF16, tag="tpb")
                nc.tensor.transpose(pt[:], gate[:, r * 128:(r + 1) * 128],
                                    identb[:])
                nc.vector.tensor_copy(gTc[:, r, :], pt[:])
            mA_bf = big.tile([128, N], BF16)
            nc.vector.tensor_tensor(mA_bf[:],
                                    gTc[:].rearrange("p r b -> p (r b)"),
                                    adjT_bf[:].rearrange("p r b -> p (r b)"),
                                    op=mybir.AluOpType.mult)
            tT_ps = [pla.tile([128, 128], F32, tag=f"tT{i}",
                              name=f"tT_ps{i}") for i in range(2)]
            for r in range(NC):
                nc.tensor.matmul(
                    tT_ps[0][:], x_bf[:, r, :],
                    mA_bf[:, r * 128:(r + 1) * 128],
                    start=(r == 0), stop=False)

            # received halves: ONE fat [128p x 1KB] load per half of the
            # A2A output in its natural packed layout, sigmoid, mask against
            # the host-packed adj replica, 4 matmuls against the x replica
            for h in range(2):
                natf = big.tile([128, 4, 128], BF16, name=f"natf{h}")
                nat_v = a2a_out[h][:].rearrange("(p k) i -> p k i", k=4)
                nc.sync.dma_start(natf[0:64, :, :], nat_v[0:64])
                nc.gpsimd.dma_start(natf[64:128, :, :], nat_v[64:128])
                sigf = big.tile([128, 4, 128], BF16, name=f"sigf{h}")
                nc.scalar.activation(
                    sigf[:].rearrange("p k i -> p (k i)"),
                    natf[:].rearrange("p k i -> p (k i)"),
                    mybir.ActivationFunctionType.Sigmoid,
                    bias=ib2b[:], scale=invb128[:])
                mB = big.tile([128, 4, 128], BF16, name=f"mB{h}")
                nc.vector.tensor_tensor(
                    mB[:].rearrange("p k i -> p (k i)"),
                    sigf[:].rearrange("p k i -> p (k i)"),
                    a_fat[h][:].rearrange("p k i -> p (k i)"),
                    op=mybir.AluOpType.mult)
                for k in range(4):
                    nc.tensor.matmul(
                        tT_ps[h][:], x_fat[h][:, k, :], mB[:, k, :],
                        start=(h == 1 and k == 0), stop=(k == 3))
                if h == 0:
                    # h0's share of tT is final: start the hg contraction
                    # while h1's AllToAll is still in flight
                    tT0 = big.tile([128, 128], BF16)
                    nc.vector.tensor_copy(tT0[:], tT_ps[0][:])
            tT1 = big.tile([128, 128], BF16)
            nc.vector.tensor_copy(tT1[:], tT_ps[1][:])

            # hgT = relu(Wg1h^T @ (tT0+tT1)); pooled partial via ACT accum
            hg_ps = psm.tile([H, 128], F32, tag="sm")
            nc.tensor.matmul(hg_ps[:], wg1h[:], tT0[:], start=True, stop=False)
            nc.tensor.matmul(hg_ps[:], wg1h[:], tT1[:], start=False, stop=True)
            hgT = big.tile([H, 128], F32)
            pooled = big.tile([H, 1], F32)
            nc.scalar.activation(hgT[:], hg_ps[:],
                                 mybir.ActivationFunctionType.Relu,
                                 accum_out=pooled[:])

            # local class logits, produced directly as [8, C] rows for the
            # logit A2A: replicate pooled into 8 columns (ACT copy with
            # per-partition scale), one matmul against scaled Wg2
            poolrep = big.tile([H, NC], F32)
            nc.scalar.mul(poolrep[:], ones64_8[:], pooled[:])
            lg8_ps = psm.tile([NC, C], F32, tag="sm")
            nc.tensor.matmul(lg8_ps[:], poolrep[:], wg2s[:])

            # ACT exp-table preload, gated on hgT so it runs during the
            # logit exchange (relu lives in every table; no reload later)
            dexp = big.tile([1, 8], F32)
            nc.scalar.activation(dexp[:], hgT[0:1, 0:8],
                                 mybir.ActivationFunctionType.Exp)

            lg8 = big.tile([NC, C], F32)
            nc.vector.tensor_copy(lg8[:], lg8_ps[:])

            # tiny AllToAll = allgather of per-core logits ([1,C] per rank)
            lga_in = dram.tile([NC, C], F32)
            nc.gpsimd.dma_start(lga_in[:], lg8[:])
            lga_out = dram.tile([NC, C], F32)
            nc.gpsimd.collective_compute(
                "AllToAll", mybir.AluOpType.bypass,
                replica_groups=[list(range(NC))],
                ins=[lga_in[:].opt()], outs=[lga_out[:].opt()])
            z8 = big.tile([NC, C], F32)
            nc.sync.dma_start(z8[:], lga_out[:])
            z_ps = psm.tile([1, C], F32, tag="sm")
            nc.tensor.matmul(z_ps[:], ones8[:], z8[:])

            # softmax on [1, 8] (logits are O(1): skip the max-subtraction);
            # exp reads the summed logits straight from PSUM
            e = big.tile([1, C], F32)
            ssum = big.tile([1, 1], F32)
            nc.scalar.activation(e[:], z_ps[:],
                                 mybir.ActivationFunctionType.Exp,
                                 accum_out=ssum[:])
            rinv = big.tile([1, 1], F32)
            nc.vector.reciprocal(rinv[:], ssum[:])
            sm = big.tile([1, C], F32)
            nc.vector.tensor_scalar(out=sm[:], in0=e[:], scalar1=rinv[:],
                                    scalar2=None, op0=mybir.AluOpType.mult)
            nc.sync.dma_start(out_dram[:], sm[:])

            if DEBUG_OUTPUTS:
                pf = big.tile([R, N], F32)
                nc.vector.tensor_copy(pf[:], pre[:])
                nc.sync.dma_start(dbg["d_pre"][:], pf[:])
                gf = big.tile([R, N], F32)
                nc.vector.tensor_copy(gf[:], gate[:])
                nc.sync.dma_start(dbg["d_gate"][:], gf[:])
                tf = big.tile([128, 128], F32)
                nc.vector.tensor_copy(tf[:], tT[:])
                nc.sync.dma_start(dbg["d_tT"][:], tf[:])
                nc.sync.dma_start(dbg["d_pooled"][:].rearrange("o h -> h o"),
                                  pooled[:])

    nc.compile()
    return nc


_NC_CACHE = None
_RUNNER_CACHE = None


def _get_nc():
    global _NC_CACHE
    if _NC_CACHE is None:
        _NC_CACHE = build()
    return _NC_CACHE


def _get_runner():
    """Cached jitted 8-core executable (run_bass_via_pjrt rebuilds the jit
    wrapper every call, costing ~300ms of host time per invocation)."""
    global _RUNNER_CACHE
    if _RUNNER_CACHE is not None:
        return _RUNNER_CACHE
    import jax
    from jax.sharding import Mesh, PartitionSpec
    from jax.experimental.shard_map import shard_map
    from concourse import mybir as mb
    from concourse.bass2jax import (_bass_exec_p, install_neuronx_cc_hook,
                                    partition_id_tensor)

    nc = _get_nc()
    install_neuronx_cc_hook()
    partition_name = (nc.partition_id_tensor.name
                      if nc.partition_id_tensor else None)
    in_names, out_names, out_avals, zero_outs = [], [], [], []
    for alloc in nc.m.functions[0].allocations:
        if not isinstance(alloc, mb.MemoryLocationSet):
            continue
        name = alloc.memorylocations[0].name
        if alloc.kind == "ExternalInput":
            if name == partition_name:
                continue
            in_names.append(name)
        elif alloc.kind == "ExternalOutput":
            shape = tuple(alloc.tensor_shape)
            dtype = mb.dt.np(alloc.dtype)
            out_names.append(name)
            out_avals.append(jax.core.ShapedArray(shape, dtype))
            zero_outs.append(np.zeros(shape, dtype))
    n_params = len(in_names)
    all_in = in_names + out_names
    if partition_name is not None:
        all_in = all_in + [partition_name]

    def _body(*args):
        operands = list(args)
        if partition_name is not None:
            operands.append(partition_id_tensor())
        outs = _bass_exec_p.bind(
            *operands,
            out_avals=tuple(out_avals),
            in_names=tuple(all_in),
            out_names=tuple(out_names),
            lowering_input_output_aliases=(),
            sim_require_finite=True,
            sim_require_nnan=True,
            nc=nc,
        )
        return tuple(outs)

    devices = jax.devices()[:NC]
    mesh = Mesh(np.asarray(devices), ("core",))
    n_outs = len(out_names)
    sharded = jax.jit(
        shard_map(_body, mesh=mesh,
                  in_specs=(PartitionSpec("core"),) * (n_params + n_outs),
                  out_specs=(PartitionSpec("core"),) * n_outs,
                  check_rep=False),
        donate_argnums=tuple(range(n_params, n_params + n_outs)),
        keep_unused=True)

    def run(in_maps):
        concat_in = [
            np.concatenate([np.asarray(in_maps[c][nm]) for c in range(NC)],
                           axis=0)
            for nm in in_names
        ]
        concat_zeros = [
            np.zeros((NC * z.shape[0], *z.shape[1:]), z.dtype)
            for z in zero_outs
        ]
        out_arrs = sharded(*concat_in, *concat_zeros)
        return [
            {nm: np.asarray(out_arrs[i]).reshape(NC, *out_avals[i].shape)[c]
             for i, nm in enumerate(out_names)}
            for c in range(NC)
        ]

    _RUNNER_CACHE = run
    return run


def kernel(**inputs):
    x = np.ascontiguousarray(np.asarray(inputs["x"], dtype=np.float32))
    embed = np.ascontiguousarray(np.asarray(inputs["embed"], dtype=np.float32))
    adj = np.ascontiguousarray(np.asarray(inputs["adj"], dtype=np.float32))
    tmp = np.asarray(inputs["tmp"], dtype=np.float32).reshape(1, 1)
    noise = np.asarray(inputs["noise"], dtype=np.float32).reshape(N, N)
    W1 = np.ascontiguousarray(np.asarray(inputs["W1"], dtype=np.float32))
    b1 = np.asarray(inputs["b1"], dtype=np.float32).reshape(1, H)
    W2 = np.ascontiguousarray(np.asarray(inputs["W2"], dtype=np.float32))
    b2 = np.asarray(inputs["b2"], dtype=np.float32).reshape(1, 1)
    Wg1 = np.ascontiguousarray(np.asarray(inputs["Wg1"], dtype=np.float32))
    Wg2 = np.ascontiguousarray(np.asarray(inputs["Wg2"], dtype=np.float32))

    in_maps = build_in_maps(x, embed, adj, noise, tmp, W1, b1, W2, b2, Wg1, Wg2)
    try:
        results = _get_runner()(in_maps)
        return np.asarray(results[0]["out"], dtype=np.float32).reshape(1, C)
    except Exception:
        nc = _get_nc()
        res = run_bass_kernel_spmd(nc, in_maps, core_ids=list(range(NC)))
        return np.asarray(res.results[0]["out"],
                          dtype=np.float32).reshape(1, C)


def build_in_maps(x, embed, adj, noise, tmp, W1, b1, W2, b2, Wg1, Wg2):
    embT = np.ascontiguousarray(embed.T)
    bf = ml_dtypes.bfloat16
    # std [j-partition] layouts: [jl, r, .] = value at global row j=128r+jl
    xbf = np.ascontiguousarray(
        x.reshape(NC, 128, F).transpose(1, 0, 2).astype(bf))
    jmaps = [_fat_jmap(h) for h in range(2)]  # [128, 4] global j indices
    xfat = np.ascontiguousarray(
        np.stack([x[jm] for jm in jmaps]).astype(bf))  # [2, 128, 4, F]
    in_maps = []
    for c in range(NC):
        sl = slice(c * R, (c + 1) * R)
        adj_slab = adj[sl]                       # [R, N] = adj[own i', j]
        adjT = np.ascontiguousarray(             # [jl, r, i'] = adj[i', j]
            adj_slab.T.reshape(NC, 128, R).transpose(1, 0, 2).astype(bf))
        afat = np.ascontiguousarray(np.stack(    # [2, 128(p), 4(k), 128(i')]
            [adj_slab[:, jm].transpose(1, 2, 0) for jm in jmaps]).astype(bf))
        in_maps.append({
            "embT_in": embT,
            "embTs_in": np.ascontiguousarray(embT[:, sl]),
            "xbf_in": xbf,
            "adjT_in": adjT,
            "xfat_in": xfat,
            "afat_in": afat,
            "noise_slab": np.ascontiguousarray(noise[sl]),
            "tmp_in": tmp,
            "w1_in": W1,
            "b1_in": b1,
            "w2_in": W2,
            "b2_in": b2,
            "wg1_in": Wg1,
            "wg2_in": Wg2,
        })
    return in_maps
